# revision 13
# baseline (speedup 1.0000x reference)
"""Bass/Trainium2 8-core kernel for nn_MultiHeadAttention_43155831390829 (v3).

Sharding: core c -> (batch b = c//4, head group g = c%4 i.e. heads 4g..4g+3).
Each core:
  - K^T / Q^T projections with a k-outer loop (all 8 PSUM banks hold the
    [256, 2048] output while x/w chunks stream in),
  - V projection rt-major with a ones-column per head (softmax denominator
    falls out of the P@V matmul),
  - causal attention processed QUERY-CHUNK-major (256-q chunks, 4 heads per
    chunk, heads pair-interleaved on disjoint PE row groups),
  - normalization off the PE critical path: pv rows copied to SBUF
    immediately (PSUM freed), denominators collected via DMA into a [4, 256]
    tile, ONE batched DVE reciprocal per chunk, partition-broadcast done by
    a 0-stride-AP DMA, fused DVE multiply into the bf16 staging tile,
  - FOUR incremental AllToAll collectives (after chunks 0-3, 4-5, 6, 7); the
    gpsimd queue only carries collective triggers + cc_out-dependent DMAs so
    collectives never stall the attention pipeline,
  - out-projection per A2A round, results DMA'd to outT (store issue
    deferred past the next collective trigger); host transposes/stitches.
"""

import sys

sys.path.insert(0, "/opt/trn_rl_repo")

import ml_dtypes
import numpy as np

import concourse.bass as bass
import concourse.mybir as mybir
import concourse.tile as tile
from concourse import bacc
from concourse.bass_utils import run_bass_kernel_spmd

N_CORES = 8
HIDDEN = 1024
HEADS = 16
HEAD_DIM = 64
BSZ = 2
SEQ = 2048
SCALE = HEAD_DIM ** (-0.5)
LOCAL_HEADS = 4
LOCAL_INNER = LOCAL_HEADS * HEAD_DIM  # 256
NCH = 8
CHQ = 256
ROUNDS = [(0, 2), (2, 4), (4, 6), (6, 8)]
SLIV = [64, 64, 64, 64]
OFFS = [0, 128, 256, 384]
OUTPROJ_AT = [4, 5, 7, 8]  # chunk index after which round r's out-proj is emitted

DT = mybir.dt.bfloat16
F32 = mybir.dt.float32
BF16 = ml_dtypes.bfloat16

_CACHED_NC = None


def build_nc(debug=False):
    nc = bacc.Bacc("TRN2", target_bir_lowering=False, debug=False, num_devices=N_CORES)

    xqT = nc.dram_tensor("xqT", [HIDDEN, SEQ], DT, kind="ExternalInput")
    xkT = nc.dram_tensor("xkT", [HIDDEN, SEQ], DT, kind="ExternalInput")
    xvT = nc.dram_tensor("xvT", [HIDDEN, SEQ], DT, kind="ExternalInput")
    wq = nc.dram_tensor("wq", [HIDDEN, LOCAL_INNER], DT, kind="ExternalInput")
    wk = nc.dram_tensor("wk", [HIDDEN, LOCAL_INNER], DT, kind="ExternalInput")
    wv = nc.dram_tensor("wv", [HIDDEN, LOCAL_INNER], DT, kind="ExternalInput")
    wo = nc.dram_tensor("wo", [HIDDEN, HIDDEN], DT, kind="ExternalInput")
    masks = nc.dram_tensor("masks", [128, 512], DT, kind="ExternalInput")
    outT = nc.dram_tensor("outT", [HIDDEN, 2 * 256], F32, kind="ExternalOutput")
    if debug:
        dbgK = nc.dram_tensor("dbgK", [256, SEQ], DT, kind="ExternalOutput")
        dbgQ = nc.dram_tensor("dbgQ", [256, SEQ], DT, kind="ExternalOutput")
        dbgS = nc.dram_tensor("dbgS", [256, 256], DT, kind="ExternalOutput")
        dbgk = nc.dram_tensor("dbgk", [2048, 128], DT, kind="ExternalOutput")

    cc_in = [nc.dram_tensor(f"cc_in{r}", [2048, SLIV[r]], DT) for r in range(4)]
    cc_out = [nc.dram_tensor(f"cc_out{r}", [2048, SLIV[r]], DT) for r in range(4)]
    cc_wu_in = nc.dram_tensor("cc_wu_in", [8, 16], DT)
    cc_wu_out = nc.dram_tensor("cc_wu_out", [8, 16], DT)

    round_of_chunk = {}
    for r, (lo, hi) in enumerate(ROUNDS):
        for ci in range(lo, hi):
            round_of_chunk[ci] = r

    with tile.TileContext(nc) as tc:
        with (
            tc.tile_pool(name="const", bufs=1) as cp,
            tc.tile_pool(name="work", bufs=3) as wp,
            tc.tile_pool(name="eps", bufs=2) as ep,
        ):
            mask_sb = cp.tile([128, 512], DT, tag="mask")
            nc.scalar.dma_start(mask_sb[:, :], masks[:, :])
            wo_sb = [cp.tile([128, HIDDEN], DT, tag=f"wo{k}", name=f"wo_sb{k}") for k in range(8)]
            kT_sb = [cp.tile([128, SEQ], DT, tag=f"kT{i}", name=f"kT_sb{i}") for i in range(2)]
            qT_sb = [cp.tile([128, SEQ], DT, tag=f"qT{i}", name=f"qT_sb{i}") for i in range(2)]
            v_sb = [cp.tile([128, LOCAL_HEADS * 65], DT, tag=f"v{t}", name=f"v_sb{t}") for t in range(16)]

            # ---- K^T / Q^T projections, k-outer over 8 PSUM banks ----------
            def proj_T(xdram, wdram, outs, x_eng, w_eng, pfx):
                with tc.tile_pool(name=f"projw_{pfx}", bufs=1, space="PSUM") as pwp:
                    ps = {}
                    for m in range(2):
                        for n in range(4):
                            ps[m, n] = pwp.tile(
                                [128, 512], F32, tag=f"pw{m}{n}", name=f"pw_{pfx}{m}{n}"
                            )
                    for k in range(8):
                        wt = wp.tile(
                            [128, LOCAL_INNER], DT, tag="wchunk", name=f"w_{pfx}{k}", bufs=4
                        )
                        w_eng.dma_start(wt[:, :], wdram[128 * k : 128 * k + 128, :])
                        xt = wp.tile([128, SEQ], DT, tag="xchunk", name=f"x_{pfx}{k}", bufs=4)
                        if k < 2:
                            for pc in range(4):
                                x_eng.dma_start(
                                    xt[:, 512 * pc : 512 * pc + 512],
                                    xdram[128 * k : 128 * k + 128, 512 * pc : 512 * pc + 512],
                                )
                        else:
                            x_eng.dma_start(xt[:, :], xdram[128 * k : 128 * k + 128, :])
                        for m in range(2):
                            for n in range(4):
                                nc.tensor.matmul(
                                    ps[m, n][:, :],
                                    lhsT=wt[:, 128 * m : 128 * m + 128],
                                    rhs=xt[:, 512 * n : 512 * n + 512],
                                    start=(k == 0),
                                    stop=(k == 7),
                                )
                    for m in range(2):
                        for n in range(4):
                            nc.vector.tensor_copy(
                                outs[m][:, 512 * n : 512 * n + 512], ps[m, n][:, :]
                            )

            # tiny warm-up AllToAll: pays the first-collective latency during
            # the projection phase instead of on round 0's critical path
            wu = wp.tile([8, 16], DT, tag="wu", bufs=1)
            nc.gpsimd.memset(wu[:, :], 0.0)
            nc.gpsimd.dma_start(cc_wu_in[:, :], wu[:, :])
            nc.gpsimd.collective_compute(
                "AllToAll",
                mybir.AluOpType.bypass,
                replica_groups=[list(range(N_CORES))],
                ins=[cc_wu_in.ap().opt()],
                outs=[cc_wu_out.ap().opt()],
            )
            # V inputs land first; V-proj in its own 2-bank pool so attention
            # never waits on the last-arriving input tensor
            xv_sb = []
            for k in range(8):
                xt = cp.tile([128, SEQ], DT, tag=f"xv{k}", name=f"xv_sb{k}")
                nc.sync.dma_start(xt[:, :], xvT[128 * k : 128 * k + 128, :])
                xv_sb.append(xt)
            wv_sb = []
            for k in range(8):
                wt = cp.tile([128, LOCAL_INNER], DT, tag=f"wv{k}", name=f"wv_sb{k}")
                nc.scalar.dma_start(wt[:, :], wv[128 * k : 128 * k + 128, :])
                wv_sb.append(wt)
            with tc.tile_pool(name="ps_v", bufs=2, space="PSUM") as pvv:
                for rt in range(16):
                    ps = pvv.tile([128, LOCAL_INNER], F32, tag="vproj", name=f"vps{rt}")
                    for k in range(8):
                        nc.tensor.matmul(
                            ps[:, :],
                            lhsT=xv_sb[k][:, 128 * rt : 128 * rt + 128],
                            rhs=wv_sb[k][:, :],
                            start=(k == 0),
                            stop=(k == 7),
                        )
                    nc.vector.memset(v_sb[rt][:, :], 1.0)
                    nc.vector.tensor_copy(
                        v_sb[rt][:, :].rearrange("p (h x) -> p h x", x=65)[:, :, 0:64],
                        ps[:, :].rearrange("p (h x) -> p h x", x=64),
                    )

            proj_T(xkT, wk, kT_sb, nc.sync, nc.scalar, "k")
            proj_T(xqT, wq, qT_sb, nc.scalar, nc.scalar, "q")

            for k in range(8):
                nc.scalar.dma_start(wo_sb[k][:, :], wo[128 * k : 128 * k + 128, :])
            if debug:
                for i in range(2):
                    nc.sync.dma_start(dbgK[128 * i : 128 * i + 128, :], kT_sb[i][:, :])
                    nc.sync.dma_start(dbgQ[128 * i : 128 * i + 128, :], qT_sb[i][:, :])

            with (
                tc.tile_pool(name="ps_st", bufs=2, space="PSUM") as pst,
                tc.tile_pool(name="ps_pv", bufs=3, space="PSUM") as ppv,
                tc.tile_pool(name="ps_o", bufs=1, space="PSUM") as pjo,
            ):
                # ---- attention, chunk-major over all 4 heads ---------------
                pending_round = []  # A2A triggered, out-proj not yet emitted

                def emit_outproj(r):
                    sliv = SLIV[r]
                    ccout_b = cc_out[r].ap().rearrange("(b p) q -> b p q", p=1024)
                    agx = []
                    for rr in range(8):
                        t_ = wp.tile(
                            [128, 2 * sliv], DT, tag=f"agx{rr}", name=f"agx{r}_{rr}", bufs=2
                        )
                        R = 256 * (rr // 2) + 128 * (rr % 2)
                        nc.sync.dma_start(
                            t_[:, :].rearrange("p (b q) -> p b q", b=2),
                            ccout_b[:, R : R + 128, :].transpose([1, 0, 2]),
                        )
                        agx.append(t_)
                    for m in range(8):
                        ps = pjo.tile(
                            [128, 2 * SLIV[0]], F32, tag="proj", name=f"ob{r}_{m}"
                        )
                        for kk in range(8):
                            nc.tensor.matmul(
                                ps[:, 0 : 2 * sliv],
                                lhsT=wo_sb[kk][:, 128 * m : 128 * m + 128],
                                rhs=agx[kk][:, :],
                                start=(kk == 0),
                                stop=(kk == 7),
                            )
                        ob = wp.tile(
                            [128, 2 * sliv], F32, tag="ob", name=f"obs{r}_{m}", bufs=2
                        )
                        nc.vector.tensor_copy(ob[:, :], ps[:, 0 : 2 * sliv])
                        nc.sync.dma_start(
                            outT[128 * m : 128 * m + 128, OFFS[r] : OFFS[r] + 2 * sliv],
                            ob[:, :],
                        )

                for ci in range(NCH):
                    r = round_of_chunk[ci]
                    sliv = SLIV[r]
                    nb = CHQ // sliv
                    lo, hi = ROUNDS[r]
                    d0 = (ci - lo) * nb
                    cc_ap = cc_in[r].ap().rearrange("(d p) q -> d p q", p=256)
                    attn = {}
                    coll = ep.tile([4, CHQ], DT, tag="coll", name=f"coll{ci}")
                    for pair in ((0, 1), (2, 3)):
                        pvt = {}
                        for hp in pair:
                            pvt[hp] = ppv.tile([65, CHQ], F32, tag="pv", name=f"pv{ci}_{hp}")
                        items = list(range(2 * ci + 2))
                        groups = [items[i : i + 4] for i in range(0, len(items), 4)]
                        for gi, g in enumerate(groups):
                            st = {}
                            for hp in pair:
                                st[hp] = pst.tile(
                                    [128, 256 * len(g)], F32, tag="st",
                                    name=f"st{ci}_{gi}_{hp}",
                                )
                            for j, t in enumerate(g):
                                for hp in pair:
                                    ti, poff = hp // 2, 64 * (hp % 2)
                                    nc.tensor.matmul(
                                        st[hp][:, 256 * j : 256 * j + 256],
                                        lhsT=kT_sb[ti][poff : poff + 64, 128 * t : 128 * t + 128],
                                        rhs=qT_sb[ti][poff : poff + 64, 256 * ci : 256 * ci + 256],
                                        start=True,
                                        stop=True,
                                    )
                            for hp in pair:
                                pT = wp.tile(
                                    [128, 256 * len(g)], DT, tag=f"pT{hp % 2}",
                                    name=f"pT{ci}_{gi}_{hp}", bufs=4,
                                )
                                nc.scalar.activation(
                                    pT[:, :], st[hp][:, :],
                                    mybir.ActivationFunctionType.Exp, scale=SCALE,
                                )
                                for j, t in enumerate(g):
                                    if t >= 2 * ci:
                                        moff = 0 if t == 2 * ci else 256
                                        nc.gpsimd.tensor_tensor(
                                            pT[:, 256 * j : 256 * j + 256],
                                            pT[:, 256 * j : 256 * j + 256],
                                            mask_sb[:, moff : moff + 256],
                                            op=mybir.AluOpType.mult,
                                        )
                                for j, t in enumerate(g):
                                    nc.tensor.matmul(
                                        pvt[hp][:, :],
                                        lhsT=v_sb[t][:, 65 * hp : 65 * hp + 65],
                                        rhs=pT[:, 256 * j : 256 * j + 256],
                                        start=(t == 0),
                                        stop=(t == 2 * ci + 1),
                                    )
                        # evacuate PSUM immediately: attn rows + den row
                        for hp in pair:
                            pv = pvt[hp]
                            at = wp.tile(
                                [65, CHQ], DT, tag=f"at{hp % 2}", name=f"at{ci}_{hp}", bufs=3
                            )
                            nc.vector.tensor_copy(at[:, :], pv[:, :])
                            attn[hp] = at
                            nc.sync.dma_start(coll[hp : hp + 1, :], at[64:65, :])
                    # one batched reciprocal for the chunk's 4 denominators
                    rcp = ep.tile([4, CHQ], DT, tag="rcp", name=f"rcp{ci}")
                    with nc.allow_low_precision(reason="bf16 softmax recip"):
                        nc.vector.reciprocal(rcp[:, :], coll[:, :])
                    for hp in range(4):
                        rcp0 = ep.tile(
                            [1, CHQ], DT, tag=f"rc{hp}", name=f"rc{ci}_{hp}"
                        )
                        nc.gpsimd.dma_start(rcp0[0:1, :], rcp[hp : hp + 1, :])
                        rcpb = ep.tile(
                            [64, CHQ], DT, tag=f"rb{hp % 2}", name=f"rb{ci}_{hp}"
                        )
                        nc.gpsimd.partition_broadcast(
                            rcpb[:, :], rcp0[0:1, :], channels=64
                        )
                        stg = wp.tile(
                            [64, CHQ], DT, tag=f"stg{hp % 2}", name=f"stg{ci}_{hp}"
                        )
                        nc.vector.tensor_tensor(
                            stg[:, :], attn[hp][0:64, :], rcpb[:, :], op=mybir.AluOpType.mult
                        )
                        nc.sync.dma_start(
                            cc_ap[d0 : d0 + nb, 64 * hp : 64 * hp + 64, :].transpose([1, 0, 2]),
                            stg[:, :].rearrange("p (d q) -> p d q", d=nb),
                        )
                        if debug and ci == 0:
                            nc.sync.dma_start(dbgS[64 * hp : 64 * hp + 64, :], stg[:, :])

                    # ---- A2A trigger at round boundaries (trigger FIRST) -------
                    if ci == hi - 1:
                        nc.gpsimd.collective_compute(
                            "AllToAll",
                            mybir.AluOpType.bypass,
                            replica_groups=[list(range(N_CORES))],
                            ins=[cc_in[r].ap().opt()],
                            outs=[cc_out[r].ap().opt()],
                        )
                        if debug and r == 0:
                            nc.gpsimd.dma_start(dbgk[:, :], cc_in[0][:, :])
                    # ---- emit out-projection for rounds whose A2A is done ------
                    for rd, tgt in enumerate(OUTPROJ_AT):
                        if tgt == ci:
                            emit_outproj(rd)
                for rd, tgt in enumerate(OUTPROJ_AT):
                    if tgt >= NCH:
                        emit_outproj(rd)

    nc.compile()
    return nc


def _make_masks():
    l = np.arange(128)[:, None]
    qr = np.arange(256)[None, :]
    m0 = np.where(l <= qr, 1.0, 0.0)
    m1 = np.where(l + 128 <= qr, 1.0, 0.0)
    return np.concatenate([m0, m1], axis=1).astype(BF16)  # [128, 512]


def make_in_maps(query, key, value, w_q, w_k, w_v, w_o):
    masks = _make_masks()
    xT = {
        n: [np.ascontiguousarray(np.asarray(x)[b].T).astype(BF16) for b in range(BSZ)]
        for n, x in (("xqT", query), ("xkT", key), ("xvT", value))
    }
    wsl = {
        n: [
            np.ascontiguousarray(
                np.asarray(w)[:, LOCAL_INNER * g : LOCAL_INNER * (g + 1)]
            ).astype(BF16)
            for g in range(4)
        ]
        for n, w in (("wq", w_q), ("wk", w_k), ("wv", w_v))
    }
    wo_bf = np.ascontiguousarray(np.asarray(w_o)).astype(BF16)
    in_maps = []
    for c in range(N_CORES):
        b, g = c // 4, c % 4
        in_maps.append(
            {
                "xqT": xT["xqT"][b],
                "xkT": xT["xkT"][b],
                "xvT": xT["xvT"][b],
                "wq": wsl["wq"][g],
                "wk": wsl["wk"][g],
                "wv": wsl["wv"][g],
                "wo": wo_bf,
                "masks": masks,
            }
        )
    return in_maps


def assemble_output(results):
    out = np.empty((BSZ, SEQ, HIDDEN), dtype=np.float32)
    for c in range(N_CORES):
        o = results[c]["outT"]  # [1024, 512]
        for r in range(4):
            sliv = SLIV[r]
            q0 = 256 * ROUNDS[r][0] + c * sliv
            for b in range(BSZ):
                cols = slice(OFFS[r] + b * sliv, OFFS[r] + (b + 1) * sliv)
                out[b, q0 : q0 + sliv, :] = o[:, cols].T
    return out


def kernel(query, key, value, w_q, w_k, w_v, w_o):
    global _CACHED_NC
    if _CACHED_NC is None:
        _CACHED_NC = build_nc()
    in_maps = make_in_maps(query, key, value, w_q, w_k, w_v, w_o)
    res = run_bass_kernel_spmd(_CACHED_NC, in_maps, core_ids=list(range(N_CORES)))
    return assemble_output(res.results)


# revision 14
# speedup vs baseline: 1.9410x; 1.9410x over previous
"""Bass/Trainium2 8-core kernel for nn_MultiHeadAttention_43155831390829 (v3).

Sharding: core c -> (batch b = c//4, head group g = c%4 i.e. heads 4g..4g+3).
Each core:
  - K^T / Q^T projections with a k-outer loop (all 8 PSUM banks hold the
    [256, 2048] output while x/w chunks stream in),
  - V projection rt-major with a ones-column per head (softmax denominator
    falls out of the P@V matmul),
  - causal attention processed QUERY-CHUNK-major (256-q chunks, 4 heads per
    chunk, heads pair-interleaved on disjoint PE row groups),
  - normalization off the PE critical path: pv rows copied to SBUF
    immediately (PSUM freed), denominators collected via DMA into a [4, 256]
    tile, ONE batched DVE reciprocal per chunk, partition-broadcast done by
    a 0-stride-AP DMA, fused DVE multiply into the bf16 staging tile,
  - FOUR incremental AllToAll collectives (after chunks 0-3, 4-5, 6, 7); the
    gpsimd queue only carries collective triggers + cc_out-dependent DMAs so
    collectives never stall the attention pipeline,
  - out-projection per A2A round, results DMA'd to outT (store issue
    deferred past the next collective trigger); host transposes/stitches.
"""

import sys

sys.path.insert(0, "/opt/trn_rl_repo")

import ml_dtypes
import numpy as np

import concourse.bass as bass
import concourse.mybir as mybir
import concourse.tile as tile
from concourse import bacc
from concourse.bass_utils import run_bass_kernel_spmd

N_CORES = 8
HIDDEN = 1024
HEADS = 16
HEAD_DIM = 64
BSZ = 2
SEQ = 2048
SCALE = HEAD_DIM ** (-0.5)
LOCAL_HEADS = 4
LOCAL_INNER = LOCAL_HEADS * HEAD_DIM  # 256
NCH = 8
CHQ = 256
ROUNDS = [(0, 2), (2, 4), (4, 6), (6, 8)]
SLIV = [64, 64, 64, 64]
OFFS = [0, 128, 256, 384]
OUTPROJ_AT = [4, 5, 7, 8]  # chunk index after which round r's out-proj is emitted

DT = mybir.dt.bfloat16
F32 = mybir.dt.float32
BF16 = ml_dtypes.bfloat16

_CACHED_NC = None


def build_nc(debug=False):
    nc = bacc.Bacc("TRN2", target_bir_lowering=False, debug=False, num_devices=N_CORES)

    xqT = nc.dram_tensor("xqT", [HIDDEN, SEQ], DT, kind="ExternalInput")
    xkT = nc.dram_tensor("xkT", [HIDDEN, SEQ], DT, kind="ExternalInput")
    xvT = nc.dram_tensor("xvT", [HIDDEN, SEQ], DT, kind="ExternalInput")
    wq = nc.dram_tensor("wq", [HIDDEN, LOCAL_INNER], DT, kind="ExternalInput")
    wk = nc.dram_tensor("wk", [HIDDEN, LOCAL_INNER], DT, kind="ExternalInput")
    wv = nc.dram_tensor("wv", [HIDDEN, LOCAL_INNER], DT, kind="ExternalInput")
    wo = nc.dram_tensor("wo", [HIDDEN, HIDDEN], DT, kind="ExternalInput")
    masks = nc.dram_tensor("masks", [128, 512], DT, kind="ExternalInput")
    outT = nc.dram_tensor("outT", [HIDDEN, 2 * 256], F32, kind="ExternalOutput")
    if debug:
        dbgK = nc.dram_tensor("dbgK", [256, SEQ], DT, kind="ExternalOutput")
        dbgQ = nc.dram_tensor("dbgQ", [256, SEQ], DT, kind="ExternalOutput")
        dbgS = nc.dram_tensor("dbgS", [256, 256], DT, kind="ExternalOutput")
        dbgk = nc.dram_tensor("dbgk", [2048, 128], DT, kind="ExternalOutput")

    cc_in = [nc.dram_tensor(f"cc_in{r}", [2048, SLIV[r]], DT) for r in range(4)]
    cc_out = [nc.dram_tensor(f"cc_out{r}", [2048, SLIV[r]], DT) for r in range(4)]
    cc_wu_in = nc.dram_tensor("cc_wu_in", [8, 16], DT)
    cc_wu_out = nc.dram_tensor("cc_wu_out", [8, 16], DT)

    round_of_chunk = {}
    for r, (lo, hi) in enumerate(ROUNDS):
        for ci in range(lo, hi):
            round_of_chunk[ci] = r

    with tile.TileContext(nc) as tc:
        with (
            tc.tile_pool(name="const", bufs=1) as cp,
            tc.tile_pool(name="work", bufs=3) as wp,
            tc.tile_pool(name="eps", bufs=2) as ep,
        ):
            mask_sb = cp.tile([128, 512], DT, tag="mask")
            nc.scalar.dma_start(mask_sb[:, :], masks[:, :])
            wo_sb = [cp.tile([128, HIDDEN], DT, tag=f"wo{k}", name=f"wo_sb{k}") for k in range(8)]
            kT_sb = [cp.tile([128, SEQ], DT, tag=f"kT{i}", name=f"kT_sb{i}") for i in range(2)]
            qT_sb = [cp.tile([128, SEQ], DT, tag=f"qT{i}", name=f"qT_sb{i}") for i in range(2)]
            v_sb = [cp.tile([128, LOCAL_HEADS * 65], DT, tag=f"v{t}", name=f"v_sb{t}") for t in range(16)]

            # ---- K^T / Q^T projections, k-outer over 8 PSUM banks ----------
            def proj_T(xdram, wdram, outs, x_eng, w_eng, pfx):
                with tc.tile_pool(name=f"projw_{pfx}", bufs=1, space="PSUM") as pwp:
                    ps = {}
                    for m in range(2):
                        for n in range(4):
                            ps[m, n] = pwp.tile(
                                [128, 512], F32, tag=f"pw{m}{n}", name=f"pw_{pfx}{m}{n}"
                            )
                    for k in range(8):
                        wt = wp.tile(
                            [128, LOCAL_INNER], DT, tag="wchunk", name=f"w_{pfx}{k}", bufs=4
                        )
                        w_eng.dma_start(wt[:, :], wdram[128 * k : 128 * k + 128, :])
                        xt = wp.tile([128, SEQ], DT, tag="xchunk", name=f"x_{pfx}{k}", bufs=4)
                        if k < 2:
                            for pc in range(4):
                                x_eng.dma_start(
                                    xt[:, 512 * pc : 512 * pc + 512],
                                    xdram[128 * k : 128 * k + 128, 512 * pc : 512 * pc + 512],
                                )
                        else:
                            x_eng.dma_start(xt[:, :], xdram[128 * k : 128 * k + 128, :])
                        for m in range(2):
                            for n in range(4):
                                nc.tensor.matmul(
                                    ps[m, n][:, :],
                                    lhsT=wt[:, 128 * m : 128 * m + 128],
                                    rhs=xt[:, 512 * n : 512 * n + 512],
                                    start=(k == 0),
                                    stop=(k == 7),
                                )
                    for m in range(2):
                        for n in range(4):
                            nc.vector.tensor_copy(
                                outs[m][:, 512 * n : 512 * n + 512], ps[m, n][:, :]
                            )

            # tiny warm-up AllToAll: pays the first-collective latency during
            # the projection phase instead of on round 0's critical path
            wu = wp.tile([8, 16], DT, tag="wu", bufs=1)
            nc.gpsimd.memset(wu[:, :], 0.0)
            nc.gpsimd.dma_start(cc_wu_in[:, :], wu[:, :])
            nc.gpsimd.collective_compute(
                "AllToAll",
                mybir.AluOpType.bypass,
                replica_groups=[list(range(N_CORES))],
                ins=[cc_wu_in.ap().opt()],
                outs=[cc_wu_out.ap().opt()],
            )
            # V inputs land first; V-proj in its own 2-bank pool so attention
            # never waits on the last-arriving input tensor
            xv_sb = []
            for k in range(8):
                xt = cp.tile([128, SEQ], DT, tag=f"xv{k}", name=f"xv_sb{k}")
                nc.sync.dma_start(xt[:, :], xvT[128 * k : 128 * k + 128, :])
                xv_sb.append(xt)
            wv_sb = []
            for k in range(8):
                wt = cp.tile([128, LOCAL_INNER], DT, tag=f"wv{k}", name=f"wv_sb{k}")
                nc.scalar.dma_start(wt[:, :], wv[128 * k : 128 * k + 128, :])
                wv_sb.append(wt)
            with tc.tile_pool(name="ps_v", bufs=2, space="PSUM") as pvv:
                for rt in range(16):
                    ps = pvv.tile([128, LOCAL_INNER], F32, tag="vproj", name=f"vps{rt}")
                    for k in range(8):
                        nc.tensor.matmul(
                            ps[:, :],
                            lhsT=xv_sb[k][:, 128 * rt : 128 * rt + 128],
                            rhs=wv_sb[k][:, :],
                            start=(k == 0),
                            stop=(k == 7),
                        )
                    nc.vector.memset(v_sb[rt][:, :], 1.0)
                    nc.vector.tensor_copy(
                        v_sb[rt][:, :].rearrange("p (h x) -> p h x", x=65)[:, :, 0:64],
                        ps[:, :].rearrange("p (h x) -> p h x", x=64),
                    )

            proj_T(xkT, wk, kT_sb, nc.sync, nc.scalar, "k")
            proj_T(xqT, wq, qT_sb, nc.scalar, nc.scalar, "q")

            for k in range(8):
                nc.scalar.dma_start(wo_sb[k][:, :], wo[128 * k : 128 * k + 128, :])
            if debug:
                for i in range(2):
                    nc.sync.dma_start(dbgK[128 * i : 128 * i + 128, :], kT_sb[i][:, :])
                    nc.sync.dma_start(dbgQ[128 * i : 128 * i + 128, :], qT_sb[i][:, :])

            with (
                tc.tile_pool(name="ps_st", bufs=2, space="PSUM") as pst,
                tc.tile_pool(name="ps_pv", bufs=3, space="PSUM") as ppv,
                tc.tile_pool(name="ps_o", bufs=1, space="PSUM") as pjo,
            ):
                # ---- attention, chunk-major over all 4 heads ---------------
                pending_round = []  # A2A triggered, out-proj not yet emitted

                def emit_outproj(r):
                    sliv = SLIV[r]
                    ccout_b = cc_out[r].ap().rearrange("(b p) q -> b p q", p=1024)
                    agx = []
                    for rr in range(8):
                        t_ = wp.tile(
                            [128, 2 * sliv], DT, tag=f"agx{rr}", name=f"agx{r}_{rr}", bufs=2
                        )
                        R = 256 * (rr // 2) + 128 * (rr % 2)
                        nc.sync.dma_start(
                            t_[:, :].rearrange("p (b q) -> p b q", b=2),
                            ccout_b[:, R : R + 128, :].transpose([1, 0, 2]),
                        )
                        agx.append(t_)
                    for m in range(8):
                        ps = pjo.tile(
                            [128, 2 * SLIV[0]], F32, tag="proj", name=f"ob{r}_{m}"
                        )
                        for kk in range(8):
                            nc.tensor.matmul(
                                ps[:, 0 : 2 * sliv],
                                lhsT=wo_sb[kk][:, 128 * m : 128 * m + 128],
                                rhs=agx[kk][:, :],
                                start=(kk == 0),
                                stop=(kk == 7),
                            )
                        ob = wp.tile(
                            [128, 2 * sliv], F32, tag="ob", name=f"obs{r}_{m}", bufs=2
                        )
                        nc.vector.tensor_copy(ob[:, :], ps[:, 0 : 2 * sliv])
                        nc.sync.dma_start(
                            outT[128 * m : 128 * m + 128, OFFS[r] : OFFS[r] + 2 * sliv],
                            ob[:, :],
                        )

                for ci in range(NCH):
                    r = round_of_chunk[ci]
                    sliv = SLIV[r]
                    nb = CHQ // sliv
                    lo, hi = ROUNDS[r]
                    d0 = (ci - lo) * nb
                    cc_ap = cc_in[r].ap().rearrange("(d p) q -> d p q", p=256)
                    attn = {}
                    coll = ep.tile([4, CHQ], DT, tag="coll", name=f"coll{ci}")
                    for pair in ((0, 1), (2, 3)):
                        pvt = {}
                        for hp in pair:
                            pvt[hp] = ppv.tile([65, CHQ], F32, tag="pv", name=f"pv{ci}_{hp}")
                        items = list(range(2 * ci + 2))
                        groups = [items[i : i + 4] for i in range(0, len(items), 4)]
                        for gi, g in enumerate(groups):
                            st = {}
                            for hp in pair:
                                st[hp] = pst.tile(
                                    [128, 256 * len(g)], F32, tag="st",
                                    name=f"st{ci}_{gi}_{hp}",
                                )
                            for j, t in enumerate(g):
                                for hp in pair:
                                    ti, poff = hp // 2, 64 * (hp % 2)
                                    nc.tensor.matmul(
                                        st[hp][:, 256 * j : 256 * j + 256],
                                        lhsT=kT_sb[ti][poff : poff + 64, 128 * t : 128 * t + 128],
                                        rhs=qT_sb[ti][poff : poff + 64, 256 * ci : 256 * ci + 256],
                                        start=True,
                                        stop=True,
                                    )
                            for hp in pair:
                                pT = wp.tile(
                                    [128, 256 * len(g)], DT, tag=f"pT{hp % 2}",
                                    name=f"pT{ci}_{gi}_{hp}", bufs=4,
                                )
                                nc.scalar.activation(
                                    pT[:, :], st[hp][:, :],
                                    mybir.ActivationFunctionType.Exp, scale=SCALE,
                                )
                                for j, t in enumerate(g):
                                    if t >= 2 * ci:
                                        moff = 0 if t == 2 * ci else 256
                                        nc.vector.tensor_tensor(
                                            pT[:, 256 * j : 256 * j + 256],
                                            pT[:, 256 * j : 256 * j + 256],
                                            mask_sb[:, moff : moff + 256],
                                            op=mybir.AluOpType.mult,
                                        )
                                for j, t in enumerate(g):
                                    nc.tensor.matmul(
                                        pvt[hp][:, :],
                                        lhsT=v_sb[t][:, 65 * hp : 65 * hp + 65],
                                        rhs=pT[:, 256 * j : 256 * j + 256],
                                        start=(t == 0),
                                        stop=(t == 2 * ci + 1),
                                    )
                        # evacuate PSUM immediately: attn rows + den row
                        for hp in pair:
                            pv = pvt[hp]
                            at = wp.tile(
                                [65, CHQ], DT, tag=f"at{hp % 2}", name=f"at{ci}_{hp}", bufs=3
                            )
                            nc.vector.tensor_copy(at[:, :], pv[:, :])
                            attn[hp] = at
                            nc.sync.dma_start(coll[hp : hp + 1, :], at[64:65, :])
                    # one batched reciprocal for the chunk's 4 denominators
                    rcp = ep.tile([4, CHQ], DT, tag="rcp", name=f"rcp{ci}")
                    with nc.allow_low_precision(reason="bf16 softmax recip"):
                        nc.vector.reciprocal(rcp[:, :], coll[:, :])
                    for hp in range(4):
                        rcp0 = ep.tile(
                            [1, CHQ], DT, tag=f"rc{hp}", name=f"rc{ci}_{hp}"
                        )
                        nc.gpsimd.dma_start(rcp0[0:1, :], rcp[hp : hp + 1, :])
                        rcpb = ep.tile(
                            [64, CHQ], DT, tag=f"rb{hp % 2}", name=f"rb{ci}_{hp}"
                        )
                        nc.gpsimd.partition_broadcast(
                            rcpb[:, :], rcp0[0:1, :], channels=64
                        )
                        stg = wp.tile(
                            [64, CHQ], DT, tag=f"stg{hp % 2}", name=f"stg{ci}_{hp}"
                        )
                        nc.vector.tensor_tensor(
                            stg[:, :], attn[hp][0:64, :], rcpb[:, :], op=mybir.AluOpType.mult
                        )
                        nc.sync.dma_start(
                            cc_ap[d0 : d0 + nb, 64 * hp : 64 * hp + 64, :].transpose([1, 0, 2]),
                            stg[:, :].rearrange("p (d q) -> p d q", d=nb),
                        )
                        if debug and ci == 0:
                            nc.sync.dma_start(dbgS[64 * hp : 64 * hp + 64, :], stg[:, :])

                    # ---- A2A trigger at round boundaries (trigger FIRST) -------
                    if ci == hi - 1:
                        nc.gpsimd.collective_compute(
                            "AllToAll",
                            mybir.AluOpType.bypass,
                            replica_groups=[list(range(N_CORES))],
                            ins=[cc_in[r].ap().opt()],
                            outs=[cc_out[r].ap().opt()],
                        )
                        if debug and r == 0:
                            nc.gpsimd.dma_start(dbgk[:, :], cc_in[0][:, :])
                    # ---- emit out-projection for rounds whose A2A is done ------
                    for rd, tgt in enumerate(OUTPROJ_AT):
                        if tgt == ci:
                            emit_outproj(rd)
                for rd, tgt in enumerate(OUTPROJ_AT):
                    if tgt >= NCH:
                        emit_outproj(rd)

    nc.compile()
    return nc


def _make_masks():
    l = np.arange(128)[:, None]
    qr = np.arange(256)[None, :]
    m0 = np.where(l <= qr, 1.0, 0.0)
    m1 = np.where(l + 128 <= qr, 1.0, 0.0)
    return np.concatenate([m0, m1], axis=1).astype(BF16)  # [128, 512]


def make_in_maps(query, key, value, w_q, w_k, w_v, w_o):
    masks = _make_masks()
    xT = {
        n: [np.ascontiguousarray(np.asarray(x)[b].T).astype(BF16) for b in range(BSZ)]
        for n, x in (("xqT", query), ("xkT", key), ("xvT", value))
    }
    wsl = {
        n: [
            np.ascontiguousarray(
                np.asarray(w)[:, LOCAL_INNER * g : LOCAL_INNER * (g + 1)]
            ).astype(BF16)
            for g in range(4)
        ]
        for n, w in (("wq", w_q), ("wk", w_k), ("wv", w_v))
    }
    wo_bf = np.ascontiguousarray(np.asarray(w_o)).astype(BF16)
    in_maps = []
    for c in range(N_CORES):
        b, g = c // 4, c % 4
        in_maps.append(
            {
                "xqT": xT["xqT"][b],
                "xkT": xT["xkT"][b],
                "xvT": xT["xvT"][b],
                "wq": wsl["wq"][g],
                "wk": wsl["wk"][g],
                "wv": wsl["wv"][g],
                "wo": wo_bf,
                "masks": masks,
            }
        )
    return in_maps


def assemble_output(results):
    out = np.empty((BSZ, SEQ, HIDDEN), dtype=np.float32)
    for c in range(N_CORES):
        o = results[c]["outT"]  # [1024, 512]
        for r in range(4):
            sliv = SLIV[r]
            q0 = 256 * ROUNDS[r][0] + c * sliv
            for b in range(BSZ):
                cols = slice(OFFS[r] + b * sliv, OFFS[r] + (b + 1) * sliv)
                out[b, q0 : q0 + sliv, :] = o[:, cols].T
    return out


def kernel(query, key, value, w_q, w_k, w_v, w_o):
    global _CACHED_NC
    if _CACHED_NC is None:
        _CACHED_NC = build_nc()
    in_maps = make_in_maps(query, key, value, w_q, w_k, w_v, w_o)
    res = run_bass_kernel_spmd(_CACHED_NC, in_maps, core_ids=list(range(N_CORES)))
    return assemble_output(res.results)
